# revision 3
# baseline (speedup 1.0000x reference)
"""CountSketch TRN2 kernel — sorted-gather reduced-K matmul (data-parallel).

out[n, b*512+k] = sum_{d: i_hash[b,d]==k} x[n,d] * s_hash[b,d] / sqrt(B)

Dense formulation (out = x @ P, P one-hot +-1) contracts over all 2048
input dims per output tile. Since each column d lands in exactly one
bucket per block, the host groups columns by (block, bucket//128): the
contraction per 128-wide output tile drops to ~512 (3.6x less PE work),
paying with a gathered bf16 copy of x per block. One-hot +-1/sqrt(8)
routing weights do the exact in-tile scatter on the PE; psum accumulates
in f32, so only input/output rounding to bf16 limits accuracy
(rel err ~2.4e-3 total).

Sharding: data-parallel over 8 NeuronCores; core i computes rows
[i*1024, (i+1)*1024); hash tables replicated (folded into the shared
routing weights).

Device pipeline per core (raw bass; TileContext multi-wait DMAs are
rejected by this toolchain's walrus):
  SP : fused [W | xg] chunk DMAs, 1.125 MiB each, 10 slots (small chunks
       keep PE idle gaps under the ~3 us HAM MID window, avoiding the
       half-clock throttle that inflated PE busy time with 2.25 MiB chunks)
  PE : per group g (32): accumulate nkt_g k-tiles into a psum pair
       (4 pairs = all 8 banks)
  DVE: psum -> bf16 stage (2 copies/group, 6 slots)
  ACT: per-group 256 KiB output DMAs
Every DMA stream uses per-slot counting semaphores so each consumer wait
names the exact transfer it needs (robust against cross-DMA completion
reordering, which corrupted the v1 dense baseline under NTFF profiling).
"""
import math
import numpy as np
import ml_dtypes
import concourse.bass as bass
from concourse import mybir
from concourse.bass_utils import run_bass_kernel_spmd

BF = ml_dtypes.bfloat16

N_CORES = 8
N_FULL = 8192
D_IN = 2048
BLOCK_SIZE = 512
N_BLOCKS = 8
QW = 128                      # bucket range per output tile (partition dim)
NQ = BLOCK_SIZE // QW         # 4 quarters per block
N_GROUPS = N_BLOCKS * NQ      # 32
M = N_FULL // N_CORES         # 1024 rows per core
MH = 512                      # psum bank = 512 f32 -> m in halves
CHUNK_T = 4                   # k-tiles per chunk DMA
WOFF = CHUNK_T * 128          # W slice leads each chunk slab
SLAB = WOFF + CHUNK_T * M     # 4608 bf16 cols per slab

CK_SLOTS = 10
PS_SLOTS = 4
ST_SLOTS = 6

_NC_CACHE = {}
_PLAN_CACHE = {}


def _plan(i_hash, s_hash):
    """Group columns by (block, bucket//128); pad groups to k-tiles of 128.

    Returns (group_kt, cols, w_packed):
      group_kt: k-tile count per group
      cols:     [KT_TOT*128] gathered column indices into x (D_IN = zero pad)
      w_packed: [128, KT_TOT*128] bf16, w_packed[p, t*128+c] = routing
                weight of k-row p in tile t to local bucket c.
    """
    inv = np.float32(1.0 / math.sqrt(N_BLOCKS))
    group_kt, cols_list, w_flat = [], [], []
    for b in range(N_BLOCKS):
        for q in range(NQ):
            d_idx = np.where(i_hash[b] // QW == q)[0]
            n = d_idx.size
            nkt = -(-n // 128)
            d_pad = np.full(nkt * 128, D_IN, np.int64)
            d_pad[:n] = d_idx
            cols_list.append(d_pad)
            W = np.zeros((nkt * 128, 128), np.float32)
            W[np.arange(n), i_hash[b, d_idx] - q * QW] = s_hash[b, d_idx] * inv
            w_flat.append(W)
            group_kt.append(nkt)
    cols = np.concatenate(cols_list)
    W = np.concatenate(w_flat, axis=0)
    kt_tot = W.shape[0] // 128
    w_packed = np.ascontiguousarray(
        W.reshape(kt_tot, 128, 128).transpose(1, 0, 2).reshape(128, kt_tot * 128)
    ).astype(BF)
    return tuple(group_kt), cols, w_packed


def _host_prep(x, cols, w_packed, kt_tot):
    """Gather + shard x into per-core fused [W | xg] chunk slabs (bf16)."""
    n_chunks = -(-kt_tot // CHUNK_T)
    kt_pad = n_chunks * CHUNK_T
    xT_ext = np.empty((D_IN + 1, N_FULL), BF)
    xT_ext[:D_IN] = x.T
    xT_ext[D_IN] = BF(0)
    xg_T = xT_ext[cols]                                    # [kt_tot*128, N]
    w_pad = np.zeros((128, kt_pad * 128), BF)
    w_pad[:, :kt_tot * 128] = w_packed
    w_chunks = w_pad.reshape(128, n_chunks, WOFF)
    shards = []
    for i in range(N_CORES):
        sl = xg_T[:, i * M:(i + 1) * M]
        arr = np.zeros((kt_pad, 128, M), BF)
        arr[:kt_tot] = sl.reshape(kt_tot, 128, M)
        xg = (arr.reshape(n_chunks, CHUNK_T, 128, M)
                 .transpose(0, 2, 1, 3)
                 .reshape(n_chunks, 128, CHUNK_T * M))
        fused = np.empty((n_chunks, 128, SLAB), BF)
        fused[:, :, :WOFF] = w_chunks.transpose(1, 0, 2)
        fused[:, :, WOFF:] = xg
        shards.append(fused)
    return shards


def _build_nc(group_kt):
    kt_tot = sum(group_kt)
    n_chunks = -(-kt_tot // CHUNK_T)
    bf = mybir.dt.bfloat16

    sched = []          # (g, kt_global, mh, start, stop, first_of_g, last_of_g)
    kt = 0
    for g, nkt in enumerate(group_kt):
        for k in range(nkt):
            for mh in range(2):
                sched.append((g, kt + k, mh,
                              k == 0, k == nkt - 1,
                              k == 0 and mh == 0,
                              k == nkt - 1 and mh == 1))
        kt += nkt
    first_of_chunk = {}
    for i, rec in enumerate(sched):
        first_of_chunk.setdefault(rec[1] // CHUNK_T, i)

    nc = bass.Bass(trn_type="TRN2", target_bir_lowering=False, debug=False)

    xg_d = nc.dram_tensor("xg", [n_chunks, 128, SLAB], bf,
                          kind="ExternalInput").ap()
    out_d = nc.dram_tensor("outT", [N_GROUPS, 128, M], bf,
                           kind="ExternalOutput").ap()

    ck = [nc.alloc_sbuf_tensor(f"ck{s}", [128, SLAB], bf).ap()
          for s in range(CK_SLOTS)]
    stage = [nc.alloc_sbuf_tensor(f"st{s}", [128, M], bf).ap()
             for s in range(ST_SLOTS)]
    ps = [[nc.alloc_psum_tensor(f"ps{s}_{h}", [128, MH], mybir.dt.float32).ap()
           for h in range(2)] for s in range(PS_SLOTS)]

    with (
        nc.semaphore("ck0") as ck0, nc.semaphore("ck1") as ck1,
        nc.semaphore("ck2") as ck2, nc.semaphore("ck3") as ck3,
        nc.semaphore("ck4") as ck4, nc.semaphore("ck5") as ck5,
        nc.semaphore("ck6") as ck6, nc.semaphore("ck7") as ck7,
        nc.semaphore("ck8") as ck8, nc.semaphore("ck9") as ck9,
        nc.semaphore("pe_ck") as pe_ck,       # +1 when PE done with chunk c
        nc.semaphore("pe_g") as pe_g,         # +1 when PE done with group g
        nc.semaphore("drain") as drain,       # +1 per DVE psum copy
        nc.semaphore("ot0") as ot0, nc.semaphore("ot1") as ot1,
        nc.semaphore("ot2") as ot2, nc.semaphore("ot3") as ot3,
        nc.semaphore("ot4") as ot4, nc.semaphore("ot5") as ot5,
        nc.Block() as block,
    ):
        ck_sems = [ck0, ck1, ck2, ck3, ck4, ck5, ck6, ck7, ck8, ck9]
        ot_sems = [ot0, ot1, ot2, ot3, ot4, ot5]

        @block.sync
        def _(sync):
            for c in range(n_chunks):
                if c >= CK_SLOTS:
                    sync.wait_ge(pe_ck, c - CK_SLOTS + 1)
                w = WOFF + min(CHUNK_T, kt_tot - c * CHUNK_T) * M
                sync.dma_start(
                    ck[c % CK_SLOTS][:, :w], xg_d[c][:, :w]
                ).then_inc(ck_sems[c % CK_SLOTS], 16)

        @block.tensor
        def _(tensor):
            waited_chunk = -1
            inc_at = {first_of_chunk[c + 1]: c
                      for c in range(n_chunks - CK_SLOTS)}
            for i, (g, ktg, mh, mm_start, mm_stop, fg, lg) in enumerate(sched):
                c = ktg // CHUNK_T
                if c > waited_chunk:
                    tensor.wait_ge(ck_sems[c % CK_SLOTS],
                                   16 * (c // CK_SLOTS + 1))
                    waited_chunk = c
                if fg and g >= PS_SLOTS:
                    tensor.wait_ge(drain, 2 * (g - PS_SLOTS + 1))
                off = ktg % CHUNK_T
                slab = ck[c % CK_SLOTS]
                mm = nc.tensor.matmul(
                    ps[g % PS_SLOTS][mh],
                    lhsT=slab[:, off * 128:(off + 1) * 128],
                    rhs=slab[:, WOFF + off * M + mh * MH:
                             WOFF + off * M + (mh + 1) * MH],
                    start=mm_start,
                    stop=mm_stop,
                )
                if lg:
                    mm.then_inc(pe_g, 1)
                elif i in inc_at:
                    mm.then_inc(pe_ck, 1)

        @block.vector
        def _(vector):
            for g in range(N_GROUPS):
                s2 = g % ST_SLOTS
                vector.wait_ge(pe_g, g + 1)
                if g >= ST_SLOTS:
                    vector.wait_ge(ot_sems[s2], 16 * (g // ST_SLOTS))
                st = stage[s2]
                nc.vector.tensor_copy(
                    st[:, :MH], ps[g % PS_SLOTS][0]).then_inc(drain, 1)
                nc.vector.tensor_copy(
                    st[:, MH:], ps[g % PS_SLOTS][1]).then_inc(drain, 1)

        @block.scalar
        def _(scalar):
            for g in range(N_GROUPS):
                scalar.wait_ge(drain, 2 * (g + 1))
                scalar.dma_start(
                    out_d[g], stage[g % ST_SLOTS]
                ).then_inc(ot_sems[g % ST_SLOTS], 16)
            for s in range(ST_SLOTS):
                n_dma = len([g for g in range(N_GROUPS) if g % ST_SLOTS == s])
                scalar.wait_ge(ot_sems[s], 16 * n_dma)

    return nc


def kernel(x, s_hash, i_hash, *, _trace=False, **_ignored):
    x = np.asarray(x, dtype=np.float32)
    s_hash = np.asarray(s_hash, dtype=np.float32)
    i_hash = np.asarray(i_hash)

    pkey = hash((i_hash.tobytes(), s_hash.tobytes()))
    if pkey not in _PLAN_CACHE:
        _PLAN_CACHE[pkey] = _plan(i_hash, s_hash)
    group_kt, cols, w_packed = _PLAN_CACHE[pkey]
    kt_tot = sum(group_kt)

    if group_kt not in _NC_CACHE:
        _NC_CACHE[group_kt] = _build_nc(group_kt)
    nc = _NC_CACHE[group_kt]

    shards = _host_prep(x, cols, w_packed, kt_tot)
    in_maps = [{"xg": shards[i]} for i in range(N_CORES)]
    res = run_bass_kernel_spmd(nc, in_maps, list(range(N_CORES)), trace=_trace)

    out = np.empty((N_FULL, N_BLOCKS * BLOCK_SIZE), dtype=np.float32)
    for i in range(N_CORES):
        r = np.asarray(res.results[i]["outT"])             # [32,128,1024]
        out[i * M:(i + 1) * M, :] = (
            r.reshape(N_GROUPS * 128, M).T.astype(np.float32))
    if _trace:
        kernel.last_exec_time_ns = res.exec_time_ns
        kernel.last_results = res
    return out
